# revision 21
# baseline (speedup 1.0000x reference)
"""Trainium2 Bass kernel for the BeamSearch dynamic-features + adjacency-mask
problem.

Reference computation (B=64, N=1024, F=8):
    d = data[batch_idx]
    arrive = current_time + dist_mat[current_poi_idx]          # [B, N]
    dyn_feat = 9 features of (d, arrive, current_time)          # [B, N, 9]
    mask_c  = mask * (arrive>=rise & arrive+dur<=arr & poly>0)  # [B, N]
    adj     = outer(mask_c, mask_c) * (1 - I)                   # [B, N, N]

Sharding: pure data-parallel over the batch dim, 8 batches per core on 8
NeuronCores. The host performs the index gathers (batch_idx / poi row gather)
and layout packing; each core computes its 8 batches' dyn features, mask, and
the 8 [1024,1024] outer-product adjacency planes (32 MB of output per core —
HBM-write bound).

Device layout: elementwise work runs on planes of shape [128, 64] where
partition p = n % 128 and free index bc = b*8 + (n // 128). The mask vector is
transposed (PE transpose + SBUF-to-SBUF DMA) into a single-partition row
rowAll[0, b*1024 + n] so that rank-1 matmuls (K=1, bf16 0/1 values - exact)
produce each 128x512 block of the outer product in PSUM.
"""

import numpy as np
from contextlib import ExitStack

import concourse.bacc as bacc
import concourse.mybir as mybir
import concourse.tile as tile
from concourse import masks
from concourse.bass_utils import run_bass_kernel_spmd

F32 = mybir.dt.float32
BF16 = mybir.dt.bfloat16
FP8 = mybir.dt.float8e4
ALU = mybir.AluOpType

B, N, F = 64, 1024, 8
NCORES = 8
BS = B // NCORES          # batches per core = 8
C = N // 128              # 128-chunks per row = 8
BC = BS * C               # free width of an elementwise plane = 64
G = 4                     # row-chunks (ci) grouped per output DMA
NPAIR = C // G            # outer-loop iterations per batch

# plane order inside the packed "pln" input (chain-critical planes first
# so the mask chain can start before the full input lands)
P_DIST, P_CT, P_RISE, P_DUR, P_ARR, P_S2, P_S1, P_S0, P_SETW, P_MSK = range(10)
NPLANES = 10

TRACE = False             # test.py flips this to capture an NTFF profile
LAST_RESULT = None        # BassKernelResults of the most recent run

_NC = None


def _build_nc():
    nc = bacc.Bacc(
        "TRN2", target_bir_lowering=False, debug=False, num_devices=NCORES
    )
    pln_d = nc.dram_tensor("pln", [128, NPLANES * BC], F32, kind="ExternalInput")
    cols_d = nc.dram_tensor("cols", [128, 2], F32, kind="ExternalInput")
    dyn_d = nc.dram_tensor("dyn", [128, 9 * BC], F32, kind="ExternalOutput")
    adj_d = nc.dram_tensor("adj", [BS, N, N], FP8, kind="ExternalOutput")

    with tile.TileContext(nc) as tc, ExitStack() as ctx:
        cpool = ctx.enter_context(tc.tile_pool(name="cpool", bufs=1))
        tpool = ctx.enter_context(tc.tile_pool(name="tpool", bufs=1))
        pspool = ctx.enter_context(tc.tile_pool(name="pspool", bufs=3, space="PSUM"))
        ptpool = ctx.enter_context(tc.tile_pool(name="ptpool", bufs=1, space="PSUM"))
        rbpool = ctx.enter_context(tc.tile_pool(name="rbpool", bufs=4))
        obpool = ctx.enter_context(tc.tile_pool(name="obpool", bufs=6))

        def ctile(shape, dtype, tg):
            return cpool.tile(shape, dtype, name=tg, tag=tg)

        def ttile(tg):
            return tpool.tile([128, BC], F32, name=tg, tag=tg)

        # ---- inputs + constants ----
        # SWDGE (gpsimd) for the inputs: the SP HWDGE ring is busy with
        # framework table loads at kernel start, delaying first-byte by ~4us.
        pln_t = ctile([128, NPLANES * BC], F32, "pln_t")
        nc.gpsimd.dma_start(pln_t[:, 0:2 * BC], pln_d[:, 0:2 * BC])
        cols_t = ctile([128, 2], F32, "cols_t")
        nc.gpsimd.dma_start(cols_t[:], cols_d[:])
        nc.gpsimd.dma_start(pln_t[:, 2 * BC:8 * BC], pln_d[:, 2 * BC:8 * BC])
        nc.gpsimd.dma_start(pln_t[:, 8 * BC:], pln_d[:, 8 * BC:])

        # warm-up: absorb engine cold-start (iram fetch) while inputs DMA in
        warm = ctile([128, 32], F32, "warm")
        nc.vector.memset(warm[:], 0.0)
        nc.vector.memset(warm[:], 1.0)
        nc.scalar.mul(warm[:], warm[:], 2.0)

        eyec = ctile([128, 128], FP8, "eyec")     # 1 - I
        nc.gpsimd.memset(eyec[:], 1.0)
        nc.gpsimd.affine_select(
            out=eyec[:], in_=eyec[:], compare_op=ALU.not_equal,
            fill=0.0, base=0, pattern=[[-1, 128]], channel_multiplier=1,
        )
        ident = ctile([128, 128], BF16, "ident")  # PE-transpose identity
        masks.make_identity(nc, ident[:])
        ones_bf = ctile([1, 128], BF16, "ones_bf")  # lhsT for row broadcast
        nc.gpsimd.memset(ones_bf[:], 1.0)

        def P(k):
            return pln_t[:, k * BC:(k + 1) * BC]

        rise, setw, dur, arr = P(P_RISE), P(P_SETW), P(P_DUR), P(P_ARR)
        s2, s1, s0, dist = P(P_S2), P(P_S1), P(P_S0), P(P_DIST)
        ct, msk = P(P_CT), P(P_MSK)
        inv = cols_t[:, 0:1]
        tst = cols_t[:, 1:2]

        # ---- mask chain (emitted first: the PE loop depends on it) ----
        arrive = ttile("arrive")
        nc.vector.tensor_add(arrive[:], ct, dist)
        c1 = ttile("c1")
        nc.vector.tensor_tensor(c1[:], arrive[:], rise, ALU.is_ge)
        t8 = ttile("t8")
        nc.vector.tensor_add(t8[:], arrive[:], dur)
        c2 = ttile("c2")
        nc.vector.tensor_tensor(c2[:], t8[:], arr, ALU.is_le)
        aa = ttile("aa")
        nc.vector.tensor_mul(aa[:], arrive[:], arrive[:])
        pa = ttile("pa")
        nc.vector.tensor_mul(pa[:], s2, aa[:])
        pb = ttile("pb")
        nc.vector.tensor_mul(pb[:], s1, arrive[:])
        pc = ttile("pc")
        nc.vector.tensor_add(pc[:], pa[:], pb[:])
        poly = ttile("poly")
        nc.vector.tensor_add(poly[:], pc[:], s0)
        c3 = ttile("c3")
        nc.vector.tensor_scalar(c3[:], poly[:], 0.0, None, ALU.is_gt)
        m12 = ttile("m12")
        nc.vector.tensor_mul(m12[:], c1[:], c2[:])
        m123 = ttile("m123")
        nc.vector.tensor_mul(m123[:], m12[:], c3[:])
        mask_f = ttile("mask_f")
        nc.vector.tensor_mul(mask_f[:], m123[:], msk)
        mask_bf = tpool.tile([128, BC], BF16, name="mask_bf", tag="mask_bf")
        nc.vector.tensor_copy(mask_bf[:], mask_f[:])

        # ---- mask -> single-partition row layout ----
        # PE transpose: [128, BC] -> PSUM [BC, 128]
        ps_T = ptpool.tile([BC, 128], BF16, name="ps_T", tag="ps_T")
        nc.tensor.transpose(ps_T[:], mask_bf[:], ident[:])
        T_s = cpool.tile([BC, 128], BF16, name="T_s", tag="T_s")
        nc.vector.tensor_copy(T_s[:], ps_T[:])
        # per-b SBUF->SBUF DMA: rowAll[0, b*N + c*128 + p] = T_s[b*C + c, p]
        rowAll = ctile([1, BS * N], BF16, "rowAll")
        for b in range(BS):
            nc.sync.dma_start(
                rowAll[0:1, b * N:(b + 1) * N].rearrange(
                    "o (c p) -> o c p", p=128
                ),
                T_s[b * C:(b + 1) * C, :],
            )

        # ---- outer products: adj[b] = rowAll[b] (x) rowAll[b], diag zeroed ----
        # row broadcast via PE ones-matmul (PSUM), one ACT copy to SBUF per b;
        # then each 128-row block is one 2x-mode DVE tensor_scalar against the
        # per-partition mask column, with the diagonal block redone via a
        # fused (row * col) * (1 - I).
        def emit_batch(b):
            base = b * N
            psB = pspool.tile([128, N], F32, name="psB", tag="psB")
            for h in range(2):
                nc.tensor.matmul(
                    psB[:, h * 512:(h + 1) * 512],
                    ones_bf[:],
                    rowAll[0:1, base + h * 512: base + (h + 1) * 512],
                )
            rowb = rbpool.tile([128, N], BF16, name="rowb", tag="rowb")
            nc.scalar.copy(rowb[:], psB[:])
            # batch 0 ships per-ci (512KB) DMAs so the write stream starts
            # as soon as the first block exists; later batches ship G*512KB.
            g = 1 if b == 0 else G
            for pr in range(C // g):
                out_t = obpool.tile([128, G * N + 1024], FP8, name="out_t", tag="out_t")
                for u in range(g):
                    ci = pr * g + u
                    colv = mask_f[:, b * C + ci: b * C + ci + 1]
                    dcol = ci * 128
                    ob = u * N
                    nc.vector.tensor_scalar_mul(
                        out_t[:, ob: ob + N], rowb[:], colv
                    )
                # zero the g diagonal blocks of this tile in one in-place
                # strided multiply by (1 - I): block u sits at column
                # u*N + (pr*g+u)*128 = pr*g*128 + u*(N+128), a uniform stride.
                start = pr * g * 128
                if g == 1:
                    dview = out_t[:, start: start + 128]
                    eview = eyec[:]
                else:
                    dview = out_t[
                        :, start: start + (g - 1) * (N + 128) + (N + 128)
                    ].rearrange("p (u w) -> p u w", w=N + 128)[:, :, 0:128]
                    eview = eyec[:, None, :].to_broadcast((128, g, 128))
                nc.vector.tensor_tensor(dview, dview, eview, ALU.mult)
                nc.sync.dma_start(
                    adj_d[b, pr * g * 128:(pr + 1) * g * 128, :].rearrange(
                        "(u p) n -> p u n", u=g
                    ),
                    out_t[:, : g * N],
                )

        def emit_features():
            # dyn features: emitted after batch 0 so the dyn DMA lands
            # mid-stream instead of trailing the final adj DMA.
            dyn_t = ctile([128, 9 * BC], F32, "dyn_t")

            def D(k):
                return dyn_t[:, k * BC:(k + 1) * BC]

            def feat(k, x, y, tg):
                # D(k) = (x - y) * inv_dur
                t = ttile(tg)
                nc.any.tensor_sub(t[:], x, y)
                nc.any.tensor_scalar_mul(D(k), t[:], inv)

            def feat_s(k, x, tg):
                # D(k) = (x - tour_start) * inv_dur
                t = ttile(tg)
                nc.any.tensor_scalar_sub(t[:], x, tst)
                nc.any.tensor_scalar_mul(D(k), t[:], inv)

            feat(0, ct, rise, "f0")
            feat(1, setw, ct, "f1")
            feat(2, arr, ct, "f2")
            feat_s(3, ct, "f3")
            feat_s(4, arrive[:], "f4")
            feat(5, arrive[:], rise, "f5")
            feat(6, setw, arrive[:], "f6")
            feat(7, arr, arrive[:], "f7")
            nc.any.tensor_scalar_mul(D(8), poly[:], 0.01)
            nc.sync.dma_start(dyn_d[:], dyn_t[:])

        emit_batch(0)
        emit_features()
        for b in range(1, BS):
            emit_batch(b)

    nc.compile()
    return nc


def _get_nc():
    global _NC
    if _NC is None:
        _NC = _build_nc()
    return _NC


def _prep_plane(a):
    """[BS, N] f32 -> [128, BC] with plane[p, b*C + c] = a[b, c*128 + p]."""
    return a.reshape(BS, C, 128).transpose(2, 0, 1).reshape(128, BC)


def kernel(data, current_time, current_poi_idx, dist_mat, batch_idx, mask):
    global LAST_RESULT
    data = np.asarray(data, dtype=np.float32)
    current_time = np.asarray(current_time, dtype=np.float32)
    dist_mat = np.asarray(dist_mat, dtype=np.float32)
    mask = np.asarray(mask, dtype=np.float32)
    batch_idx = np.asarray(batch_idx)
    current_poi_idx = np.asarray(current_poi_idx)

    d = data[batch_idx]                                   # [B, N, F]
    dist_rows = dist_mat[current_poi_idx]                 # [B, N]
    ct_b = np.broadcast_to(current_time.reshape(B, 1), (B, N))

    tour_start = data[0, 0, 0]
    max_dur = np.float32(data[0, 0, 3] - tour_start)
    inv_dur = np.float32(np.float32(1.0) / max_dur)

    cols = np.empty((128, 2), np.float32)
    cols[:, 0] = inv_dur
    cols[:, 1] = tour_start

    in_maps = []
    for i in range(NCORES):
        s = slice(i * BS, (i + 1) * BS)
        # order must match P_DIST, P_CT, P_RISE, P_DUR, P_ARR, P_S2, P_S1,
        # P_S0, P_SETW, P_MSK
        planes = [
            _prep_plane(dist_rows[s]),
            _prep_plane(np.ascontiguousarray(ct_b[s])),
            _prep_plane(np.ascontiguousarray(d[s, :, 0])),   # rise
            _prep_plane(np.ascontiguousarray(d[s, :, 2])),   # dur
            _prep_plane(np.ascontiguousarray(d[s, :, 3])),   # arr
            _prep_plane(np.ascontiguousarray(d[s, :, 4])),   # s2
            _prep_plane(np.ascontiguousarray(d[s, :, 5])),   # s1
            _prep_plane(np.ascontiguousarray(d[s, :, 6])),   # s0
            _prep_plane(np.ascontiguousarray(d[s, :, 1])),   # setw
            _prep_plane(mask[s]),
        ]
        pln = np.ascontiguousarray(np.concatenate(planes, axis=1))
        in_maps.append({"pln": pln, "cols": cols})

    nc = _get_nc()
    res = run_bass_kernel_spmd(
        nc, in_maps, core_ids=list(range(NCORES)), trace=TRACE
    )
    LAST_RESULT = res

    dyn_parts, adj_parts = [], []
    for i in range(NCORES):
        dyn_c = res.results[i]["dyn"].reshape(128, 9, BS, C)
        dyn_parts.append(
            np.ascontiguousarray(dyn_c.transpose(2, 3, 0, 1)).reshape(BS, N, 9)
        )
        adj_parts.append(res.results[i]["adj"])
    dyn_feat = np.concatenate(dyn_parts, axis=0)
    # adj is computed on-device in fp8 (e4m3): every value is exactly 0.0 or
    # 1.0, both exactly representable, so the upcast to f32 is lossless.
    adj = np.concatenate(adj_parts, axis=0).astype(np.float32)
    return dyn_feat, adj


# revision 22
# speedup vs baseline: 1.2658x; 1.2658x over previous
"""Trainium2 Bass kernel for the BeamSearch dynamic-features + adjacency-mask
problem.

Reference computation (B=64, N=1024, F=8):
    d = data[batch_idx]
    arrive = current_time + dist_mat[current_poi_idx]          # [B, N]
    dyn_feat = 9 features of (d, arrive, current_time)          # [B, N, 9]
    mask_c  = mask * (arrive>=rise & arrive+dur<=arr & poly>0)  # [B, N]
    adj     = outer(mask_c, mask_c) * (1 - I)                   # [B, N, N]

Sharding: pure data-parallel over the batch dim, 8 batches per core on 8
NeuronCores. The host performs the index gathers (batch_idx / poi row gather)
and layout packing; each core computes its 8 batches' dyn features, mask, and
the 8 [1024,1024] outer-product adjacency planes (32 MB of output per core —
HBM-write bound).

Device layout: elementwise work runs on planes of shape [128, 64] where
partition p = n % 128 and free index bc = b*8 + (n // 128). The mask vector is
transposed (PE transpose + SBUF-to-SBUF DMA) into a single-partition row
rowAll[0, b*1024 + n] so that rank-1 matmuls (K=1, bf16 0/1 values - exact)
produce each 128x512 block of the outer product in PSUM.
"""

import numpy as np
from contextlib import ExitStack

import concourse.bacc as bacc
import concourse.mybir as mybir
import concourse.tile as tile
from concourse import masks
from concourse.bass_utils import run_bass_kernel_spmd

F32 = mybir.dt.float32
BF16 = mybir.dt.bfloat16
FP8 = mybir.dt.float8e4
ALU = mybir.AluOpType

B, N, F = 64, 1024, 8
NCORES = 8
BS = B // NCORES          # batches per core = 8
C = N // 128              # 128-chunks per row = 8
BC = BS * C               # free width of an elementwise plane = 64
G = 4                     # row-chunks (ci) grouped per output DMA
NPAIR = C // G            # outer-loop iterations per batch

# plane order inside the packed "pln" input (chain-critical planes first
# so the mask chain can start before the full input lands)
P_DIST, P_CT, P_RISE, P_DUR, P_ARR, P_S2, P_S1, P_S0, P_SETW, P_MSK = range(10)
NPLANES = 10

TRACE = False             # test.py flips this to capture an NTFF profile
LAST_RESULT = None        # BassKernelResults of the most recent run

_NC = None


def _build_nc():
    nc = bacc.Bacc(
        "TRN2", target_bir_lowering=False, debug=False, num_devices=NCORES
    )
    pln_d = nc.dram_tensor("pln", [128, NPLANES * BC], F32, kind="ExternalInput")
    cols_d = nc.dram_tensor("cols", [128, 2], F32, kind="ExternalInput")
    dyn_d = nc.dram_tensor("dyn", [128, 9 * BC], F32, kind="ExternalOutput")
    adj_d = nc.dram_tensor("adj", [BS, N, N], FP8, kind="ExternalOutput")

    with tile.TileContext(nc) as tc, ExitStack() as ctx:
        cpool = ctx.enter_context(tc.tile_pool(name="cpool", bufs=1))
        tpool = ctx.enter_context(tc.tile_pool(name="tpool", bufs=1))
        pspool = ctx.enter_context(tc.tile_pool(name="pspool", bufs=3, space="PSUM"))
        ptpool = ctx.enter_context(tc.tile_pool(name="ptpool", bufs=1, space="PSUM"))
        rbpool = ctx.enter_context(tc.tile_pool(name="rbpool", bufs=4))
        obpool = ctx.enter_context(tc.tile_pool(name="obpool", bufs=6))

        def ctile(shape, dtype, tg):
            return cpool.tile(shape, dtype, name=tg, tag=tg)

        def ttile(tg):
            return tpool.tile([128, BC], F32, name=tg, tag=tg)

        # ---- inputs + constants ----
        # SWDGE (gpsimd) for the inputs: the SP HWDGE ring is busy with
        # framework table loads at kernel start, delaying first-byte by ~4us.
        pln_t = ctile([128, NPLANES * BC], F32, "pln_t")
        nc.gpsimd.dma_start(pln_t[:, 0:2 * BC], pln_d[:, 0:2 * BC])
        cols_t = ctile([128, 2], F32, "cols_t")
        nc.gpsimd.dma_start(cols_t[:], cols_d[:])
        nc.gpsimd.dma_start(pln_t[:, 2 * BC:8 * BC], pln_d[:, 2 * BC:8 * BC])
        nc.gpsimd.dma_start(pln_t[:, 8 * BC:], pln_d[:, 8 * BC:])

        # warm-up: absorb engine cold-start (iram fetch) while inputs DMA in
        warm = ctile([128, 32], F32, "warm")
        nc.vector.memset(warm[:], 0.0)
        nc.vector.memset(warm[:], 1.0)
        nc.scalar.mul(warm[:], warm[:], 2.0)

        eyec = ctile([128, 128], FP8, "eyec")     # 1 - I
        nc.gpsimd.memset(eyec[:], 1.0)
        nc.gpsimd.affine_select(
            out=eyec[:], in_=eyec[:], compare_op=ALU.not_equal,
            fill=0.0, base=0, pattern=[[-1, 128]], channel_multiplier=1,
        )
        ident = ctile([128, 128], BF16, "ident")  # PE-transpose identity
        masks.make_identity(nc, ident[:])
        ones_bf = ctile([1, 128], BF16, "ones_bf")  # lhsT for row broadcast
        nc.gpsimd.memset(ones_bf[:], 1.0)

        def P(k):
            return pln_t[:, k * BC:(k + 1) * BC]

        rise, setw, dur, arr = P(P_RISE), P(P_SETW), P(P_DUR), P(P_ARR)
        s2, s1, s0, dist = P(P_S2), P(P_S1), P(P_S0), P(P_DIST)
        ct, msk = P(P_CT), P(P_MSK)
        inv = cols_t[:, 0:1]
        tst = cols_t[:, 1:2]

        # ---- mask chain (emitted first: the PE loop depends on it) ----
        arrive = ttile("arrive")
        nc.vector.tensor_add(arrive[:], ct, dist)
        c1 = ttile("c1")
        nc.vector.tensor_tensor(c1[:], arrive[:], rise, ALU.is_ge)
        t8 = ttile("t8")
        nc.vector.tensor_add(t8[:], arrive[:], dur)
        c2 = ttile("c2")
        nc.vector.tensor_tensor(c2[:], t8[:], arr, ALU.is_le)
        aa = ttile("aa")
        nc.vector.tensor_mul(aa[:], arrive[:], arrive[:])
        pa = ttile("pa")
        nc.vector.tensor_mul(pa[:], s2, aa[:])
        pb = ttile("pb")
        nc.vector.tensor_mul(pb[:], s1, arrive[:])
        pc = ttile("pc")
        nc.vector.tensor_add(pc[:], pa[:], pb[:])
        poly = ttile("poly")
        nc.vector.tensor_add(poly[:], pc[:], s0)
        c3 = ttile("c3")
        nc.vector.tensor_scalar(c3[:], poly[:], 0.0, None, ALU.is_gt)
        m12 = ttile("m12")
        nc.vector.tensor_mul(m12[:], c1[:], c2[:])
        m123 = ttile("m123")
        nc.vector.tensor_mul(m123[:], m12[:], c3[:])
        mask_f = ttile("mask_f")
        nc.vector.tensor_mul(mask_f[:], m123[:], msk)
        mask_bf = tpool.tile([128, BC], BF16, name="mask_bf", tag="mask_bf")
        nc.vector.tensor_copy(mask_bf[:], mask_f[:])

        # ---- mask -> single-partition row layout ----
        # PE transpose: [128, BC] -> PSUM [BC, 128]
        ps_T = ptpool.tile([BC, 128], BF16, name="ps_T", tag="ps_T")
        nc.tensor.transpose(ps_T[:], mask_bf[:], ident[:])
        T_s = cpool.tile([BC, 128], BF16, name="T_s", tag="T_s")
        nc.vector.tensor_copy(T_s[:], ps_T[:])
        # per-b SBUF->SBUF DMA: rowAll[0, b*N + c*128 + p] = T_s[b*C + c, p]
        rowAll = ctile([1, BS * N], BF16, "rowAll")
        for b in range(BS):
            nc.sync.dma_start(
                rowAll[0:1, b * N:(b + 1) * N].rearrange(
                    "o (c p) -> o c p", p=128
                ),
                T_s[b * C:(b + 1) * C, :],
            )

        # ---- outer products: adj[b] = rowAll[b] (x) rowAll[b], diag zeroed ----
        # row broadcast via PE ones-matmul (PSUM), one ACT copy to SBUF per b;
        # then each 128-row block is one 2x-mode DVE tensor_scalar against the
        # per-partition mask column, with the diagonal block redone via a
        # fused (row * col) * (1 - I).
        def emit_batch(b):
            base = b * N
            psB = pspool.tile([128, N], F32, name="psB", tag="psB")
            for h in range(2):
                nc.tensor.matmul(
                    psB[:, h * 512:(h + 1) * 512],
                    ones_bf[:],
                    rowAll[0:1, base + h * 512: base + (h + 1) * 512],
                )
            rowb = rbpool.tile([128, N], BF16, name="rowb", tag="rowb")
            nc.scalar.copy(rowb[:], psB[:])
            # batch 0 ships per-ci (512KB) DMAs so the write stream starts
            # as soon as the first block exists; later batches ship G*512KB.
            g = 1 if b == 0 else G
            for pr in range(C // g):
                out_t = obpool.tile([128, G * N + 1024], FP8, name="out_t", tag="out_t")
                for u in range(g):
                    ci = pr * g + u
                    colv = mask_f[:, b * C + ci: b * C + ci + 1]
                    dcol = ci * 128
                    ob = u * N
                    nc.any.tensor_scalar_mul(
                        out_t[:, ob: ob + N], rowb[:], colv
                    )
                # zero the g diagonal blocks of this tile in one in-place
                # strided multiply by (1 - I): block u sits at column
                # u*N + (pr*g+u)*128 = pr*g*128 + u*(N+128), a uniform stride.
                start = pr * g * 128
                if g == 1:
                    dview = out_t[:, start: start + 128]
                    eview = eyec[:]
                else:
                    dview = out_t[
                        :, start: start + (g - 1) * (N + 128) + (N + 128)
                    ].rearrange("p (u w) -> p u w", w=N + 128)[:, :, 0:128]
                    eview = eyec[:, None, :].to_broadcast((128, g, 128))
                nc.any.tensor_tensor(dview, dview, eview, ALU.mult)
                nc.sync.dma_start(
                    adj_d[b, pr * g * 128:(pr + 1) * g * 128, :].rearrange(
                        "(u p) n -> p u n", u=g
                    ),
                    out_t[:, : g * N],
                )

        def emit_features():
            # dyn features: emitted after batch 0 so the dyn DMA lands
            # mid-stream instead of trailing the final adj DMA.
            dyn_t = ctile([128, 9 * BC], F32, "dyn_t")

            def D(k):
                return dyn_t[:, k * BC:(k + 1) * BC]

            def feat(k, x, y, tg):
                # D(k) = (x - y) * inv_dur
                t = ttile(tg)
                nc.any.tensor_sub(t[:], x, y)
                nc.any.tensor_scalar_mul(D(k), t[:], inv)

            def feat_s(k, x, tg):
                # D(k) = (x - tour_start) * inv_dur
                t = ttile(tg)
                nc.any.tensor_scalar_sub(t[:], x, tst)
                nc.any.tensor_scalar_mul(D(k), t[:], inv)

            feat(0, ct, rise, "f0")
            feat(1, setw, ct, "f1")
            feat(2, arr, ct, "f2")
            feat_s(3, ct, "f3")
            feat_s(4, arrive[:], "f4")
            feat(5, arrive[:], rise, "f5")
            feat(6, setw, arrive[:], "f6")
            feat(7, arr, arrive[:], "f7")
            nc.any.tensor_scalar_mul(D(8), poly[:], 0.01)
            nc.sync.dma_start(dyn_d[:], dyn_t[:])

        emit_batch(0)
        emit_features()
        for b in range(1, BS):
            emit_batch(b)

    nc.compile()
    return nc


def _get_nc():
    global _NC
    if _NC is None:
        _NC = _build_nc()
    return _NC


def _prep_plane(a):
    """[BS, N] f32 -> [128, BC] with plane[p, b*C + c] = a[b, c*128 + p]."""
    return a.reshape(BS, C, 128).transpose(2, 0, 1).reshape(128, BC)


def kernel(data, current_time, current_poi_idx, dist_mat, batch_idx, mask):
    global LAST_RESULT
    data = np.asarray(data, dtype=np.float32)
    current_time = np.asarray(current_time, dtype=np.float32)
    dist_mat = np.asarray(dist_mat, dtype=np.float32)
    mask = np.asarray(mask, dtype=np.float32)
    batch_idx = np.asarray(batch_idx)
    current_poi_idx = np.asarray(current_poi_idx)

    d = data[batch_idx]                                   # [B, N, F]
    dist_rows = dist_mat[current_poi_idx]                 # [B, N]
    ct_b = np.broadcast_to(current_time.reshape(B, 1), (B, N))

    tour_start = data[0, 0, 0]
    max_dur = np.float32(data[0, 0, 3] - tour_start)
    inv_dur = np.float32(np.float32(1.0) / max_dur)

    cols = np.empty((128, 2), np.float32)
    cols[:, 0] = inv_dur
    cols[:, 1] = tour_start

    in_maps = []
    for i in range(NCORES):
        s = slice(i * BS, (i + 1) * BS)
        # order must match P_DIST, P_CT, P_RISE, P_DUR, P_ARR, P_S2, P_S1,
        # P_S0, P_SETW, P_MSK
        planes = [
            _prep_plane(dist_rows[s]),
            _prep_plane(np.ascontiguousarray(ct_b[s])),
            _prep_plane(np.ascontiguousarray(d[s, :, 0])),   # rise
            _prep_plane(np.ascontiguousarray(d[s, :, 2])),   # dur
            _prep_plane(np.ascontiguousarray(d[s, :, 3])),   # arr
            _prep_plane(np.ascontiguousarray(d[s, :, 4])),   # s2
            _prep_plane(np.ascontiguousarray(d[s, :, 5])),   # s1
            _prep_plane(np.ascontiguousarray(d[s, :, 6])),   # s0
            _prep_plane(np.ascontiguousarray(d[s, :, 1])),   # setw
            _prep_plane(mask[s]),
        ]
        pln = np.ascontiguousarray(np.concatenate(planes, axis=1))
        in_maps.append({"pln": pln, "cols": cols})

    nc = _get_nc()
    res = run_bass_kernel_spmd(
        nc, in_maps, core_ids=list(range(NCORES)), trace=TRACE
    )
    LAST_RESULT = res

    dyn_parts, adj_parts = [], []
    for i in range(NCORES):
        dyn_c = res.results[i]["dyn"].reshape(128, 9, BS, C)
        dyn_parts.append(
            np.ascontiguousarray(dyn_c.transpose(2, 3, 0, 1)).reshape(BS, N, 9)
        )
        adj_parts.append(res.results[i]["adj"])
    dyn_feat = np.concatenate(dyn_parts, axis=0)
    # adj is computed on-device in fp8 (e4m3): every value is exactly 0.0 or
    # 1.0, both exactly representable, so the upcast to f32 is lossless.
    adj = np.concatenate(adj_parts, axis=0).astype(np.float32)
    return dyn_feat, adj
